# revision 20
# baseline (speedup 1.0000x reference)
"""LinearSelfAttention kernel for TRN2 (8 NeuronCores, batch-parallel).

out = H + (P @ mask(H^T Q H) ...) reassociated via the Gram matrix:
    G  = H' H'^T                [257, 257]   (H' = first n=2048 columns)
    At = Q^T G (P^T / n)        [257, 257]   (= (P G Q / n)^T)
    outT = H^T + H^T At         [2049, 257]  (computed transposed, t on partitions)

Host prep supplies H in both layouts (native bf16 + transposed bf16 tiles) and
zero-pads every 257-sized matmul dimension to 128-multiples so all matmuls are
K=128/M<=128 with no tiny stationary loads (K=1 or M=1 loads carry
row_grp/col_grp tags and cannot use the background weight buffer, serializing
the PE). G[256,256] is computed off the PE (DVE column reduce + one dot).
Output is stored transposed in bf16; the host transposes back and casts.

Sharding: data-parallel over batch, 2 samples per core, P/Q replicated.
"""

import sys

sys.path.insert(0, "/opt/trn_rl_repo")

import numpy as np
import ml_dtypes

B, D1, N1 = 16, 257, 2049  # batch, d+1, n+1
N = N1 - 1  # 2048
NCORES = 8
BPC = B // NCORES  # samples per core
NT = 16  # full 128-row tiles of s; t=2048 is a 17th partial tile
TW = 257
NWARM = 26

_cached = {}


def _build():
    import concourse.bass as bass
    import concourse.tile as tile
    from concourse import bacc, mybir
    from concourse.masks import make_identity
    from concourse import bass_isa
    from contextlib import ExitStack

    f32 = mybir.dt.float32
    bf16 = mybir.dt.bfloat16
    ALU = mybir.AluOpType

    nc = bacc.Bacc("TRN2", target_bir_lowering=False, debug=False, num_devices=NCORES)

    Htb_d = nc.declare_dram_parameter("Htb", [BPC, 128, 17 * TW], bf16, isOutput=False)
    Hb_d = nc.declare_dram_parameter("Hb", [BPC, D1, N1], bf16, isOutput=False)
    CP_d = nc.declare_dram_parameter("CP", [128, 3 * 641], bf16, isOutput=False)
    YT_d = nc.declare_dram_parameter("YT", [BPC, 128, 17 * TW], bf16, isOutput=True)

    with tile.TileContext(nc) as tc:
        with ExitStack() as ctx:
            const = ctx.enter_context(tc.tile_pool(name="const", bufs=1))
            htp = ctx.enter_context(tc.tile_pool(name="htp", bufs=2))
            hbp = ctx.enter_context(tc.tile_pool(name="hbp", bufs=2))
            gcp = ctx.enter_context(tc.tile_pool(name="gcp", bufs=2))
            chp = ctx.enter_context(tc.tile_pool(name="chp", bufs=2))
            otp = ctx.enter_context(tc.tile_pool(name="otp", bufs=2))

            # ---- warmup constants first so the PE can start immediately ----
            wsb = const.tile([128, 128], bf16, tag="wsb", name="wsb")
            nc.vector.memset(wsb[:, :], 0.0)

            # ---- input DMAs ------------------------------------------------
            # Htb first: DMA semaphore lanes are allocated in emission order
            # and recycled mod-8 with issue-time waits, so the loads that gate
            # the PE must claim the first lanes.
            # htb0 splits across both HWDGE queues for full bandwidth; htb1
            # follows; each chunk's DMA is issued on an alternating queue.
            htb = []
            HTCHS = [[(0, 2), (2, 7), (9, 8)], [(0, 5), (5, 6), (11, 6)]]
            engs = [nc.sync, nc.scalar]
            ei = 0
            for b in range(BPC):
                t = htp.tile([128, 17, TW], bf16, tag="htb", name=f"htb{b}")
                for c0, cn in HTCHS[b]:
                    engs[ei % 2].dma_start(
                        t[:, c0 : c0 + cn, :], Htb_d[b, :, c0 * TW : (c0 + cn) * TW]
                    )
                    ei += 1
                htb.append(t)

            cpt3 = const.tile([128, 3, 641], bf16, tag="cpt", name="cpt3")
            nc.gpsimd.dma_start(cpt3[:, :, :], CP_d[:, :])
            cpt = [cpt3[:, kc] for kc in range(3)]

            ident = const.tile([128, 128], bf16, tag="ident", name="ident")
            make_identity(nc, ident[:, :])

            # Hb on gpsimd, gated behind the Htb loads by probe copies so the
            # SDMA engines aren't stolen from the critical Htb stream.
            probe = const.tile([128, 16], bf16, tag="probe", name="probe")
            hb = []
            for b in range(BPC):
                nc.gpsimd.tensor_copy(
                    probe[0:1, 8 * b : 8 * b + 8], htb[1][0:1, 16, 8 * b : 8 * b + 8]
                )
                t0 = hbp.tile([128, N1], bf16, tag="hb0", name=f"hb0_{b}")
                t1 = hbp.tile([128, N1], bf16, tag="hb1", name=f"hb1_{b}")
                t2 = hbp.tile([128, N1], bf16, tag="hb2", name=f"hb2_{b}")
                nc.gpsimd.memset(t2[:, :], 0.0)
                nc.gpsimd.dma_start(t0[:, :], Hb_d[b, 0:128, :])
                nc.gpsimd.dma_start(t1[:, :], Hb_d[b, 128:256, :])
                nc.gpsimd.dma_start(t2[0:1, :], Hb_d[b, 256:257, :])
                hb.append((t0, t1, t2))

            # zero-padded G tiles: [128, 384]; cols 257:384 stay zero
            gc0, gc1, gc2 = [], [], []
            for b in range(BPC):
                g0 = gcp.tile([128, 384], bf16, tag="gc0", name=f"gc0_{b}")
                g1 = gcp.tile([128, 384], bf16, tag="gc1", name=f"gc1_{b}")
                g2 = gcp.tile([128, 384], bf16, tag="gc2", name=f"gc2_{b}")
                nc.vector.memset(g0[:, 256:384], 0.0)
                nc.vector.memset(g1[:, 256:384], 0.0)
                nc.vector.memset(g2[:, :], 0.0)
                gc0.append(g0)
                gc1.append(g1)
                gc2.append(g2)

            # ---- PE warmup -------------------------------------------------
            with tc.tile_pool(name="wp", bufs=1, space="PSUM") as wp:
                wps = wp.tile([128, 512], f32, tag="wps", name="warm_ps")
                for i in range(NWARM):
                    nc.tensor.matmul(
                        wps[:, 0:128],
                        wsb[:, :],
                        wsb[:, :],
                        start=(i == 0),
                        stop=(i == NWARM - 1),
                    )

            MSL = [(0, 128), (128, 128), (256, 128)]

            with tc.tile_pool(name="gp", bufs=2, space="PSUM") as gp, tc.tile_pool(
                name="tp", bufs=2, space="PSUM"
            ) as tp, tc.tile_pool(name="cp2", bufs=3, space="PSUM") as cpp:
                gpsA = [None] * BPC
                gpsB = [None] * BPC
                wall = [None] * BPC
                m1c = [[None] * 3 for _ in range(BPC)]
                at = [[None] * 3 for _ in range(BPC)]

                def g_stage(b):
                    gpsA[b] = gp.tile([128, TW], f32, tag="gps", name=f"gpsA{b}")
                    gpsB[b] = gp.tile([128, 129], f32, tag="gps", name=f"gpsB{b}")
                    for st in range(NT):
                        nc.tensor.matmul(
                            gpsA[b][:, :],
                            htb[b][:, st, 0:128],
                            htb[b][:, st, :],
                            start=(st == 0),
                            stop=(st == NT - 1),
                        )
                        nc.tensor.matmul(
                            gpsB[b][:, :],
                            htb[b][:, st, 128:256],
                            htb[b][:, st, 128:257],
                            start=(st == 0),
                            stop=(st == NT - 1),
                        )
                    scr = chp.tile([128, 16], bf16, tag="scr", name=f"scr{b}")
                    w = chp.tile([128, 1], f32, tag="wred", name=f"w{b}")
                    nc.scalar.activation(
                        scr[:, :],
                        htb[b][:, 0:16, 256],
                        mybir.ActivationFunctionType.Square,
                        accum_out=w[:, :],
                    )
                    wb = chp.tile([128, 1], bf16, tag="wb", name=f"wb{b}")
                    nc.vector.tensor_copy(wb[:, :], w[:, :])
                    ptw = tp.tile([128, 128], bf16, tag="pt", name=f"ptw{b}")
                    nc.tensor.transpose(ptw[0:1, 0:128], wb[:, :], ident[:, :])
                    wrow = chp.tile([128, 128], bf16, tag="wrow", name=f"wrow{b}")
                    g22 = chp.tile([128, 1], f32, tag="g22", name=f"g22{b}")
                    nc.scalar.activation(
                        wrow[0:1, :],
                        ptw[0:1, 0:128],
                        mybir.ActivationFunctionType.Copy,
                        accum_out=g22[0:1, :],
                    )
                    wall[b] = g22

                def recon(b):
                    nc.scalar.copy(gc2[b][0:1, 256:257], wall[b][0:1, 0:1])
                    nc.vector.tensor_copy(gc0[b][:, 0:257], gpsA[b][:, :])
                    nc.scalar.copy(gc1[b][:, 128:257], gpsB[b][:, :])
                    pt0 = tp.tile([128, 128], bf16, tag="pt", name=f"pt0_{b}")
                    nc.tensor.transpose(pt0[:, :], gc0[b][:, 128:256], ident[:, :])
                    nc.vector.tensor_copy(gc1[b][:, 0:128], pt0[:, :])
                    pt1 = tp.tile([128, 128], bf16, tag="pt", name=f"pt1_{b}")
                    nc.tensor.transpose(
                        pt1[0:1, 0:128], gc0[b][:, 256:257], ident[:, :]
                    )
                    nc.vector.tensor_copy(gc2[b][0:1, 0:128], pt1[0:1, 0:128])
                    pt2 = tp.tile([128, 128], bf16, tag="pt", name=f"pt2_{b}")
                    nc.tensor.transpose(
                        pt2[0:1, 0:128], gc1[b][:, 256:257], ident[:, :]
                    )
                    nc.vector.tensor_copy(gc2[b][0:1, 128:256], pt2[0:1, 0:128])

                def m1_stage(b):
                    gcs = [gc0[b], gc1[b], gc2[b]]
                    for mc, (mo, msz) in enumerate(MSL):
                        p = cpp.tile([128, TW], f32, tag="chp8", name=f"m1p{b}_{mc}")
                        for kc in range(3):
                            nc.tensor.matmul(
                                p[:, :],
                                gcs[kc][:, mo : mo + msz],
                                cpt[kc][:, 384:641],
                                start=(kc == 0),
                                stop=(kc == 2),
                            )
                        t = chp.tile(
                            [128, TW], bf16, tag=f"m1c{mc}", name=f"m1c{b}_{mc}"
                        )
                        if mc == 0:
                            nc.vector.tensor_copy(t[:, :], p[:, :])
                        else:
                            nc.scalar.copy(t[:, :], p[:, :])
                        m1c[b][mc] = t

                def at_stage(b):
                    for mc, (mo, msz) in enumerate(MSL):
                        p = cpp.tile([128, TW], f32, tag="chp8", name=f"atp{b}_{mc}")
                        for kc in range(3):
                            nc.tensor.matmul(
                                p[:, :],
                                cpt[kc][:, mo : mo + msz],
                                m1c[b][kc][:, :],
                                start=(kc == 0),
                                stop=(kc == 2),
                            )
                        t = chp.tile([128, TW], bf16, tag=f"at{mc}", name=f"at{b}_{mc}")
                        if mc == 0:
                            nc.vector.tensor_copy(t[:, :], p[:, :])
                        else:
                            nc.scalar.copy(t[:, :], p[:, :])
                        at[b][mc] = t

                g_stage(0)
                g_stage(1)
                recon(0)
                m1_stage(0)
                recon(1)
                m1_stage(1)
                at_stage(0)
                at_stage(1)

            # ---- final: outT = Ht + Ht @ At (t on partitions) --------------
            STORES0 = {
                16: (16, 17, "sync"),
                6: (0, 7, "scalar"),
                13: (7, 14, "sync"),
                15: (14, 16, "scalar"),
            }
            STORES1 = {
                6: (0, 7, "scalar"),
                12: (7, 13, "sync"),
                15: (13, 16, "scalar"),
                16: (16, 17, "sync"),
            }
            with tc.tile_pool(name="fp", bufs=4, space="PSUM") as fpp:
                for b in range(BPC):
                    ot = otp.tile([128, 17, TW], bf16, tag="ot", name=f"ot{b}")
                    hbs = hb[b]
                    STORES = STORES0 if b == 0 else STORES1
                    order = [16] + list(range(NT)) if b == 0 else list(range(NT)) + [16]
                    for st in order:
                        if st < NT:
                            tsl = slice(st * 128, (st + 1) * 128)
                            rows = 128
                        else:
                            tsl = slice(2048, 2049)
                            rows = 1
                        p = fpp.tile([128, TW], f32, tag="pa", name=f"pa{b}_{st}")
                        for ec in range(3):
                            nc.tensor.matmul(
                                p[:rows, :],
                                hbs[ec][:, tsl],
                                at[b][ec][:, :],
                                start=(ec == 0),
                                stop=(ec == 2),
                            )
                        nc.vector.tensor_add(
                            ot[:rows, st, :], p[:rows, :], htb[b][:rows, st, :]
                        )
                        if st in STORES:
                            c0, c1, engname = STORES[st]
                            eng = nc.sync if engname == "sync" else nc.scalar
                            prows = 1 if st == 16 else 128
                            eng.dma_start(
                                YT_d[b, 0:prows, c0 * TW : c1 * TW],
                                ot[0:prows, c0:c1, :],
                            )

    nc.compile()
    return nc


def _prep_in_maps(H, P, Q):
    H = np.ascontiguousarray(H, dtype=np.float32)
    Hb = H.astype(ml_dtypes.bfloat16)
    HtT = np.zeros((B, 17 * 128, D1), dtype=ml_dtypes.bfloat16)
    HtT[:, :N1, :] = np.ascontiguousarray(H.transpose(0, 2, 1)).astype(
        ml_dtypes.bfloat16
    )
    Htb = np.ascontiguousarray(
        HtT.reshape(B, 17, 128, D1).transpose(0, 2, 1, 3)
    ).reshape(B, 128, 17 * D1)
    CP = np.zeros((384, 641), dtype=np.float32)
    CP[:D1, :D1] = Q
    CP[:D1, 384 : 384 + D1] = P.T / N
    CP = np.ascontiguousarray(
        CP.astype(ml_dtypes.bfloat16).reshape(3, 128, 641).transpose(1, 0, 2)
    ).reshape(128, 3 * 641)
    return [
        {
            "Htb": Htb[c * BPC : (c + 1) * BPC],
            "Hb": Hb[c * BPC : (c + 1) * BPC],
            "CP": CP,
        }
        for c in range(NCORES)
    ]


def _post(res):
    out = np.empty((B, D1, N1), dtype=np.float32)
    for c in range(NCORES):
        yt = np.asarray(res.results[c]["YT"]).astype(np.float32)
        yt = yt.reshape(BPC, 128, 17, D1).transpose(0, 2, 1, 3).reshape(BPC, -1, D1)
        out[c * BPC : (c + 1) * BPC] = yt[:, :N1, :].transpose(0, 2, 1)
    return out


def kernel(H, P, Q):
    from concourse.bass_utils import run_bass_kernel_spmd

    if "nc" not in _cached:
        _cached["nc"] = _build()
    nc = _cached["nc"]

    in_maps = _prep_in_maps(H, P, Q)
    res = run_bass_kernel_spmd(nc, in_maps, list(range(NCORES)))
    return _post(res)
